# revision 19
# baseline (speedup 1.0000x reference)
"""IoU loss kernel for Trainium2, data-parallel over 8 NeuronCores.

Math (per box, columns = x-center, y-center, half-size s):
    w  = relu(S - max(|dx|, |D|)),  S = s1+s2, D = s1-s2, dx = x1-x2
    h  = relu(S - max(|dy|, |D|))
    ov = w*h
    ue = 2S^2 + 2D^2 - ov                      (union)
    iou = ov / (ue + 1e-7)
    loss = -sum(log(iou + 1e-7));  iou_sum = sum(iou)

Design (per [128, wt]-box tile):
  DVE:  one custom ABS_DIFF op |O-T| over the full interleaved stream with a
        deinterleaving output AP -> [|dx| | |dy| | |D|] contiguous blocks
        (kills the whole separate abs stage), S into the adjacent block,
        then five f16 2x tensor_tensor ops: max / sub (2wt fused via
        broadcast APs), ov, qs, ue, d = ln(ov+c) - ln(ue+eps).
  ACT:  Relu(w~|h~) in place, Square(sqrt2*[|D| | S]) -> [2D^2|2S^2],
        Ln(ue+eps), Ln(ov+1e-9),
        Exp(d)  + accum  -> per-tile  sum(iou)       (free reduction)
        Relu(d - ln eps) + accum -> per-tile loss partial; the clamp
        max(d, ln eps) == ln(iou+eps) up to O(1e-3) relative on the sums.
  Tiles: 8 x [128, 1024] per core; per-tile DMA pairs (12KB/partition,
        contiguous), 6-deep raw buffers; the first two targets-tiles load
        via the ACT HWDGE ring so fill DMAs overlap the sync-ring ones.
  Host: sum the [128, 2*NT] partials of 8 cores in float64;
        loss = -(sum_relu + N*ln(eps)).
"""

import numpy as np

import concourse.bass as bass
import concourse.mybir as mybir
import concourse.dve_ops as dve_ops
from concourse import tile
from concourse.bass_utils import run_bass_kernel_spmd
from concourse.dve_spec import Spec, Src0, Src1, maxx, lower as dve_lower
from concourse.dve_uop import DveOpSpec

N = 8388608
NCORES = 8
NS = N // NCORES      # 1048576 boxes per core
P = 128
U = NS // P           # 8192 boxes per partition
EPS = 1e-7
CLO = 1e-9            # bias for Ln(ov + .): << eps*typical(ue)
LNEPS = float(np.log(np.float32(EPS)))
RT2 = 1.4142135623730951

# Per-core compute-tile plan (boxes per partition per tile).
PLAN = [1024] * 8
assert sum(PLAN) == U

F32 = mybir.dt.float32
F16 = mybir.dt.float16
Op = mybir.AluOpType
Act = mybir.ActivationFunctionType


def _register_absdiff() -> "dve_ops.DveOp":
    """Register |Src0 - Src1| as a custom DVE op (row past the builtin 16).

    One 1x instruction handles the fp32->f16 abs-diff of the whole
    interleaved (w c) stream; the deinterleaving is free via the output
    access pattern."""
    name = "ABS_DIFF_ANT_K"
    for o in dve_ops.OPS:
        if o.name == name:
            return o
    spec = Spec(
        body=maxx(Src0 - Src1, Src1 - Src0),
        reference=lambda in0, in1, s0, s1, imm2: np.maximum(
            in0.astype(np.float32) - in1.astype(np.float32),
            in1.astype(np.float32) - in0.astype(np.float32),
        ),
    )
    row = max(dve_ops._SUB_OPCODE_FOR_NAME.values()) + 1
    assert row < 0x20, "custom DVE opcode rows exhausted"
    shas = {}
    for ver in ("v3", "v4"):
        uops = dve_lower(spec, ver=ver)
        shas[ver] = DveOpSpec(name=name, opcode=row, uops=uops, rd1_en=True).sha(ver)
    op = dve_ops.DveOp(name, spec, False, shas)
    dve_ops.OPS.append(op)
    dve_ops.CUSTOM_DVE_SPECS[name] = spec
    dve_ops._SUB_OPCODE_FOR_NAME[name] = row
    return op


def _build(plan=None, raw_bufs: int = 6, compile_passes: bool = True,
           trace_sim: bool = False) -> bass.Bass:
    from concourse import bacc

    absdiff = _register_absdiff()
    plan = list(plan or PLAN)
    u = sum(plan)
    ns = P * u
    NT = len(plan)
    WMAX = max(plan)
    nc = bacc.Bacc()
    outs_d = nc.dram_tensor("outputs", [ns, 3], F32, kind="ExternalInput")
    tars_d = nc.dram_tensor("targets", [ns, 3], F32, kind="ExternalInput")
    acc_d = nc.dram_tensor("acc", [P, 2 * NT], F32, kind="ExternalOutput")

    # Partition p owns boxes [p*u, (p+1)*u): any column range of the
    # [P, 3u] image is a legal contiguous-per-partition DMA.
    outs_v = outs_d[:, :].rearrange("(p u) c -> p (u c)", p=P)
    tars_v = tars_d[:, :].rearrange("(p u) c -> p (u c)", p=P)

    with tile.TileContext(nc, trace_sim=trace_sim) as tc:
        with tc.tile_pool(name="main", bufs=2) as pool:
            eps_t = pool.tile([P, 1], F32, tag="eps", bufs=1)
            nc.vector.memset(eps_t[:, :], EPS)
            clo_t = pool.tile([P, 1], F32, tag="clo", bufs=1)
            nc.vector.memset(clo_t[:, :], CLO)
            nlc_t = pool.tile([P, 1], F32, tag="nlc", bufs=1)
            nc.vector.memset(nlc_t[:, :], -LNEPS)
            accs = pool.tile([P, 2 * NT], F32, tag="accs", bufs=1)

            off = 0
            for t, w in enumerate(plan):
                c0, c1 = off * 3, (off + w) * 3
                off += w
                rawO = pool.tile([P, 3 * w], F32, tag="rawO", bufs=raw_bufs)
                rawT = pool.tile([P, 3 * w], F32, tag="rawT", bufs=raw_bufs)
                nc.sync.dma_start(out=rawO[:, :], in_=outs_v[:, c0:c1])
                if t <= 1:
                    # First targets tile on the ACT HWDGE ring: the two fill
                    # DMAs run concurrently, halving pipeline-fill latency.
                    nc.scalar.dma_start(out=rawT[:, :], in_=tars_v[:, c0:c1])
                else:
                    nc.sync.dma_start(out=rawT[:, :], in_=tars_v[:, c0:c1])

                o3 = rawO.rearrange("p (w c) -> p w c", c=3)
                t3 = rawT.rearrange("p (w c) -> p w c", c=3)

                # AD3 blocks: [ |dx| | |dy| | |D| | S ]
                AD3 = pool.tile([P, 4 * w], F16, tag="AD3")
                ad4 = AD3.rearrange("p (c w) -> p c w", c=4)
                nc.vector._custom_dve(
                    absdiff,
                    out=ad4[:, 0:3, :],
                    in0=o3.transpose([0, 2, 1]),
                    in1=t3.transpose([0, 2, 1]),
                )
                nc.vector.tensor_tensor(ad4[:, 3, :], o3[:, :, 2], t3[:, :, 2], Op.add)

                # mwmh = max([|dx| | |dy|], |D| broadcast)     [P, 2, w]
                MM = pool.tile([P, 2 * w], F16, tag="MM", bufs=1)
                mm2 = MM.rearrange("p (c w) -> p c w", c=2)
                aD_rep = AD3[:, 2 * w : 3 * w].unsqueeze(1).broadcast_to([P, 2, w])
                nc.vector.tensor_tensor(mm2[:, :, :], ad4[:, 0:2, :], aD_rep, Op.max)

                # [w~ | h~] = S broadcast - mwmh
                WT = pool.tile([P, 2 * w], F16, tag="WT")
                wt2 = WT.rearrange("p (c w) -> p c w", c=2)
                S_rep = AD3[:, 3 * w : 4 * w].unsqueeze(1).broadcast_to([P, 2, w])
                nc.vector.tensor_tensor(wt2[:, :, :], S_rep, mm2[:, :, :], Op.subtract)

                # relu([w~ | h~]) in place on ACT (priority: beats the
                # previous tile's terminal Exp/ReluLoss in the ACT queue)
                with tc.high_priority(offset=16):
                    nc.scalar.activation(WT[:, :], WT[:, :], Act.Relu)

                # ov = w * h
                OV = pool.tile([P, w], F16, tag="OV")
                nc.vector.tensor_tensor(OV[:, :], WT[:, 0:w], WT[:, w : 2 * w], Op.mult)

                # QQ = Square(sqrt2 * [|D| | S]) = [2D^2 | 2S^2]
                QQ = pool.tile([P, 2 * w], F16, tag="QQ")
                with tc.high_priority(offset=16):
                    nc.scalar.activation(
                        QQ[:, :], AD3[:, 2 * w : 4 * w], Act.Square, scale=RT2
                    )
                QS = pool.tile([P, w], F16, tag="QS", bufs=1)
                nc.vector.tensor_tensor(QS[:, :], QQ[:, 0:w], QQ[:, w : 2 * w], Op.add)

                UE = pool.tile([P, w], F16, tag="UE")
                nc.vector.tensor_tensor(UE[:, :], QS[:, :], OV[:, :], Op.subtract)

                # LL = [Ln(ue+eps) | Ln(ov+c)]
                LL = pool.tile([P, 2 * w], F16, tag="LL")
                with tc.high_priority(offset=16):
                    nc.scalar.activation(
                        LL[:, 0:w], UE[:, :], Act.Ln, bias=eps_t[:, 0:1]
                    )
                    nc.scalar.activation(
                        LL[:, w : 2 * w], OV[:, :], Act.Ln, bias=clo_t[:, 0:1]
                    )

                # d = ln(ov+c) - ln(ue+eps)
                DD = pool.tile([P, w], F16, tag="DD")
                nc.vector.tensor_tensor(
                    DD[:, :], LL[:, w : 2 * w], LL[:, 0:w], Op.subtract
                )

                # iou = Exp(d); accum -> sum(iou) partial
                IOU = pool.tile([P, w], F16, tag="IOU", bufs=1)
                nc.scalar.activation(
                    IOU[:, :], DD[:, :], Act.Exp, accum_out=accs[:, t : t + 1]
                )
                # Relu(d - ln eps); accum -> sum(max(d, ln eps)) - w*ln(eps)
                LR = pool.tile([P, w], F16, tag="LR", bufs=1)
                nc.scalar.activation(
                    LR[:, :],
                    DD[:, :],
                    Act.Relu,
                    bias=nlc_t[:, 0:1],
                    accum_out=accs[:, NT + t : NT + t + 1],
                )

            nc.sync.dma_start(out=acc_d[:, :], in_=accs[:, :])

    if compile_passes:
        nc.compile()
    return nc


_NC_CACHE: list[bass.Bass] = []


def _get_nc() -> bass.Bass:
    if not _NC_CACHE:
        _NC_CACHE.append(_build())
    return _NC_CACHE[0]


def _reduce_host(results) -> tuple:
    iou_sum = 0.0
    loss_relu = 0.0
    for c in range(NCORES):
        acc = np.asarray(results[c]["acc"], dtype=np.float64)
        nt = acc.shape[1] // 2
        iou_sum += acc[:, :nt].sum()
        loss_relu += acc[:, nt:].sum()
    loss = -(loss_relu + N * LNEPS)
    return (np.float32(loss), np.float32(iou_sum))


def _run(inputs: dict, trace: bool = False, trace_kwargs: dict | None = None):
    outputs = np.ascontiguousarray(np.asarray(inputs["outputs"], dtype=np.float32))
    targets = np.ascontiguousarray(np.asarray(inputs["targets"], dtype=np.float32))
    assert outputs.shape == (N, 3) and targets.shape == (N, 3)

    nc = _get_nc()
    in_maps = [
        {
            "outputs": outputs[c * NS : (c + 1) * NS],
            "targets": targets[c * NS : (c + 1) * NS],
        }
        for c in range(NCORES)
    ]
    kw = {}
    if trace:
        kw["trace"] = True
        if trace_kwargs:
            kw["trace_kwargs"] = trace_kwargs
    res = run_bass_kernel_spmd(nc, in_maps, list(range(NCORES)), **kw)
    return _reduce_host(res.results), res


def kernel(**inputs) -> tuple:
    (loss, iou_sum), _ = _run(inputs)
    return (loss, iou_sum)
